# revision 3
# baseline (speedup 1.0000x reference)
"""Trainium2 Bass kernel for the gated equivariant MLP (gnn_message_passing).

Baseline structure (single-bank psums, proven scheduler concurrency) with
surgical upgrades:
- gate columns of fctp1's scalar path run as 3 fp8e4 DoubleRow matmuls
  (K=256 contracted at 2 rows/cell/cycle) instead of 6 bf16 matmuls;
  silu columns stay f16 x e3m4 (precision-critical, straight to o0).
- inputs are fp8: x (channel-major, de-interleaved) ships as e3m4 for the
  f16-stationary matmuls, plus an e4m3 copy of the 0e block for DoubleRow.
  Input DMA bytes drop 2x vs bf16.
- all other operands/intermediates/outputs fp16 (2^-11 mantissa) instead of
  bf16, reclaiming error budget for the fp8 stages.
- o0's bias is added on host; the out tensor is fp16 channel-major and the
  host re-transposes/interleaves (host time is off the device critical path).
- gates are computed as t=tanh(v/2) (same ACT table set as silu); z=(t+1)*y
  with host-halved fctp2 l-weights reconstructs sigmoid gating.
"""

import sys

import numpy as np
import ml_dtypes

for _p in ("/root/.axon_site/_ro/trn_rl_repo", "/root/.axon_site/_ro/pypackages",
           "/opt/trn_rl_repo", "/opt/pypackages"):
    if _p not in sys.path:
        sys.path.append(_p)

import concourse.bass as bass
import concourse.bacc as bacc
import concourse.tile as tile
from concourse import mybir
from concourse.bass_utils import run_bass_kernel_spmd

F32 = mybir.dt.float32
F16 = mybir.dt.float16
E4 = mybir.dt.float8e4
E3 = mybir.dt.float8e3
DRM = mybir.MatmulPerfMode.DoubleRow

E4np = ml_dtypes.float8_e4m3
E3np = ml_dtypes.float8_e3m4

N_CORES = 8
N_TOTAL = 65536
NPC = N_TOTAL // N_CORES  # nodes per core

CT = 512   # compute node tile (moving free dim / PSUM bank)
DT = 1024  # input DMA node tile

CFG = {"xin": 3, "mid": 2, "outp": 3, "ps_s": 2, "ps_y": 3, "ps_o": 3}


def build_program(npc=NPC, rep=1, num_devices=N_CORES, sim_safe=False,
                  loop_n=1, variant='full'):
    nc = bacc.Bacc("TRN2", target_bir_lowering=False, debug=False,
                   num_devices=num_devices)

    xt = nc.dram_tensor("xt", [960, npc], E3, kind="ExternalInput").ap()
    x0e4_d = nc.dram_tensor("x0e4", [128, 2, npc], E4, kind="ExternalInput").ap()
    w1sf_d = nc.dram_tensor("w1sf", [256, 384], F16, kind="ExternalInput").ap()
    w1sg_d = nc.dram_tensor("w1sg", [128, 2, 384], E4, kind="ExternalInput").ap()
    b1_d = nc.dram_tensor("b1", [768, 1], F32, kind="ExternalInput").ap()
    w1l1_d = nc.dram_tensor("w1l1", [128, 192], F16, kind="ExternalInput").ap()
    w1l2_d = nc.dram_tensor("w1l2", [128, 96], F16, kind="ExternalInput").ap()
    w2s_d = nc.dram_tensor("w2s", [384, 256], F16, kind="ExternalInput").ap()
    w2l1_d = nc.dram_tensor("w2l1", [192, 128], F16, kind="ExternalInput").ap()
    w2l2_d = nc.dram_tensor("w2l2", [96, 64], F16, kind="ExternalInput").ap()
    out = nc.dram_tensor("out", [960, npc], F16, kind="ExternalOutput").ap()

    with tile.TileContext(nc) as tc:
        if loop_n > 1:
            with tc.For_i(0, loop_n, 1,
                          hint_engines=(mybir.EngineType.PE,
                                        mybir.EngineType.Activation,
                                        mybir.EngineType.DVE,
                                        mybir.EngineType.SP,
                                        mybir.EngineType.Pool)):
                _emit(tc, nc, xt, x0e4_d, w1sf_d, w1sg_d, b1_d, w1l1_d,
                      w1l2_d, w2s_d, w2l1_d, w2l2_d, out, npc, rep)
        else:
            _emit(tc, nc, xt, x0e4_d, w1sf_d, w1sg_d, b1_d, w1l1_d,
                  w1l2_d, w2s_d, w2l1_d, w2l2_d, out, npc, rep)

    nc.compile()
    return nc


def _emit(tc, nc, xt, x0e4_d, w1sf_d, w1sg_d, b1_d, w1l1_d, w1l2_d,
          w2s_d, w2l1_d, w2l2_d, out, npc, rep):
    import contextlib
    ctx = contextlib.ExitStack()
    AF = mybir.ActivationFunctionType
    ADD = mybir.AluOpType.add
    MUL = mybir.AluOpType.mult
    with ctx:
        consts = ctx.enter_context(tc.tile_pool(name="consts", bufs=1))
        xin = ctx.enter_context(tc.tile_pool(name="xin", bufs=CFG["xin"]))
        mid = ctx.enter_context(tc.tile_pool(name="mid", bufs=CFG["mid"]))
        outp = ctx.enter_context(tc.tile_pool(name="outp", bufs=CFG["outp"]))
        psum = ctx.enter_context(tc.tile_pool(name="psum", bufs=2, space="PSUM"))

        # ---- constants into SBUF (once) ----
        w1sf_t = []
        for kb in range(2):
            t = consts.tile([128, 384], F16, tag=f"w1sf{kb}")
            nc.sync.dma_start(t[:], w1sf_d[kb * 128:(kb + 1) * 128, :])
            w1sf_t.append(t)
        w1sg_t = consts.tile([128, 2, 384], E4, tag="w1sg")
        nc.sync.dma_start(w1sg_t[:], w1sg_d[:, :, :])
        b1_t = []
        for mb in range(6):
            t = consts.tile([128, 1], F32, tag=f"b1_{mb}")
            nc.sync.dma_start(t[:], b1_d[mb * 128:(mb + 1) * 128, :])
            b1_t.append(t)
        w1l1_t = consts.tile([128, 192], F16, tag="w1l1")
        nc.sync.dma_start(w1l1_t[:], w1l1_d[:, :])
        w1l2_t = consts.tile([128, 96], F16, tag="w1l2")
        nc.sync.dma_start(w1l2_t[:], w1l2_d[:, :])
        w2s_t = []
        for kb in range(3):
            t = consts.tile([128, 256], F16, tag=f"w2s{kb}")
            nc.sync.dma_start(t[:], w2s_d[kb * 128:(kb + 1) * 128, :])
            w2s_t.append(t)
        w2l1a_t = consts.tile([128, 128], F16, tag="w2l1a")
        nc.sync.dma_start(w2l1a_t[:], w2l1_d[0:128, :])
        w2l1b_t = consts.tile([128, 128], F16, tag="w2l1b")
        nc.sync.dma_start(w2l1b_t[0:64, :], w2l1_d[128:192, :])
        nc.sync.dma_start(w2l1b_t[64:128, :], w2l1_d[128:192, :])
        w2l2_t = consts.tile([96, 64], F16, tag="w2l2")
        nc.sync.dma_start(w2l2_t[:], w2l2_d[:, :])

        n_dt = npc // DT
        n_ct_per_dt = DT // CT

        for _r in range(rep):
            for idt in range(n_dt):
                d0 = idt * DT
                # ---- input DMA (e3m4 main + e4m3 x0 copy) ----
                xa = xin.tile([128, 7, DT], E3, tag="xa")
                nc.sync.dma_start(
                    xa[:], xt[0:896, d0:d0 + DT].rearrange(
                        '(b p) n -> p b n', p=128))
                xbt = xin.tile([64, DT], E3, tag="xb7")
                nc.sync.dma_start(xbt[:], xt[896:960, d0:d0 + DT])
                x0e4_t = xin.tile([128, 2, DT], E4, tag="x0e4")
                nc.sync.dma_start(x0e4_t[:], x0e4_d[:, :, d0:d0 + DT])
                xb = [xa[:, cb, :] for cb in range(7)] + [xbt[:]]
                # x2 component i -> (tile, partition base)
                x2map = [(xb[5], 0), (xb[5], 64), (xb[6], 0), (xb[6], 64), (xb[7], 0)]

                for ict in range(n_ct_per_dt):
                    ns = slice(ict * CT, (ict + 1) * CT)
                    n0 = d0 + ict * CT

                    # ---- fctp1 scalar path + gate nonlinearities ----
                    sc_t = []   # 3x [128, CT] f16 silu outputs
                    g_t = []    # 3x [128, CT] f16 tanh(v/2) gates (g2: rows 0:96)
                    for bi in range(3):   # silu blocks, f16 x e3m4, K=256
                        ps = psum.tile([128, CT], F32, tag="ps_s", bufs=CFG["ps_s"])
                        c0 = bi * 128
                        for kb in range(2):
                            nc.tensor.matmul(
                                ps[:], w1sf_t[kb][:, c0:c0 + 128], xb[kb][:, ns],
                                start=(kb == 0), stop=(kb == 1))
                        dst = mid.tile([128, CT], F16, tag=f"sg{bi}")
                        nc.scalar.activation(dst[:], ps[:], AF.Silu,
                                             bias=b1_t[bi][:])
                        sc_t.append(dst)
                    for gb in range(3):   # gate blocks, fp8e4 DoubleRow, K=256
                        ps = psum.tile([128, CT], F32, tag="ps_s", bufs=CFG["ps_s"])
                        nc.tensor.matmul(
                            ps[:], w1sg_t[:, :, gb * 128:(gb + 1) * 128],
                            x0e4_t[:, :, ns], start=True, stop=True,
                            perf_mode=DRM)
                        dst = mid.tile([128, CT], F16, tag=f"sg{3 + gb}")
                        # t = tanh(v/2); host pre-halved the gate bias rows
                        nc.scalar.activation(dst[:], ps[:], AF.Tanh,
                                             bias=b1_t[3 + gb][:], scale=0.5)
                        g_t.append(dst)

                    # ---- fctp1 l=1, l=2 paths + gating: z = (t+1)*y ----
                    one = 1.0
                    z1a, z1b, z2 = [], [], []
                    for i in range(3):
                        ps = psum.tile([128, CT], F32, tag="ps_y", bufs=CFG["ps_y"])
                        nc.tensor.matmul(ps[:], w1l1_t[:, 0:128], xb[2 + i][:, ns],
                                         start=True, stop=True)
                        z = mid.tile([128, CT], F16, tag=f"z1a{i}")
                        nc.vector.scalar_tensor_tensor(
                            z[:], g_t[0][:], one, ps[:],
                            op0=ADD, op1=MUL)
                        z1a.append(z)
                    psb = psum.tile([128, CT], F32, tag="ps_y", bufs=CFG["ps_y"])
                    nc.tensor.matmul(psb[0:64, :], w1l1_t[:, 128:192],
                                     xb[2][:, ns], start=True, stop=True,
                                     tile_position=(0, 0))
                    nc.tensor.matmul(psb[64:128, :], w1l1_t[:, 128:192],
                                     xb[3][:, ns], start=True, stop=True,
                                     tile_position=(0, 64))
                    ps2b = psum.tile([64, CT], F32, tag="ps_y", bufs=CFG["ps_y"])
                    nc.tensor.matmul(ps2b[:], w1l1_t[:, 128:192], xb[4][:, ns],
                                     start=True, stop=True)
                    z1bp = mid.tile([128, CT], F16, tag="z1bp")
                    nc.vector.scalar_tensor_tensor(
                        z1bp[0:64, :], g_t[1][0:64, :], one, psb[0:64, :],
                        op0=ADD, op1=MUL)
                    nc.vector.scalar_tensor_tensor(
                        z1bp[64:128, :], g_t[1][64:128, :], one, psb[64:128, :],
                        op0=ADD, op1=MUL)
                    z1b2 = mid.tile([64, CT], F16, tag="z1b2")
                    nc.vector.scalar_tensor_tensor(
                        z1b2[:], g_t[1][0:64, :], one, ps2b[:],
                        op0=ADD, op1=MUL)
                    z1b = [z1bp[0:64, :], z1bp[64:128, :], z1b2[:]]
                    for i in range(5):
                        xt2, p0 = x2map[i]
                        ps = psum.tile([96, CT], F32, tag="ps_y", bufs=CFG["ps_y"])
                        nc.tensor.matmul(ps[:], w1l2_t[p0:p0 + 64, :],
                                         xt2[p0:p0 + 64, ns], start=True, stop=True)
                        z = mid.tile([96, CT], F16, tag=f"z2{i}")
                        nc.vector.scalar_tensor_tensor(
                            z[:], g_t[2][0:96, :], one, ps[:],
                            op0=ADD, op1=MUL)
                        z2.append(z)

                    # ---- fctp2 (weight-stationary -> channel-major out) ----
                    out_sb = outp.tile([128, 8, CT], F16, tag="out_sb")
                    for ob in range(2):
                        ps = psum.tile([128, CT], F32, tag="ps_o", bufs=CFG["ps_o"])
                        obs = slice(ob * 128, (ob + 1) * 128)
                        for kb in range(3):
                            nc.tensor.matmul(ps[:], w2s_t[kb][:, obs], sc_t[kb][:],
                                             start=(kb == 0), stop=(kb == 2))
                        nc.scalar.activation(out_sb[:, ob, :], ps[:], AF.Copy)
                    psl1 = []
                    for i in range(3):
                        ps = psum.tile([128, CT], F32, tag="ps_o", bufs=CFG["ps_o"])
                        nc.tensor.matmul(ps[:], w2l1a_t[:], z1a[i][:],
                                         start=True, stop=False)
                        psl1.append(ps)
                    nc.tensor.matmul(psl1[0][:], w2l1b_t[0:64, :], z1b[0],
                                     start=False, stop=True)
                    nc.tensor.matmul(psl1[1][:], w2l1b_t[64:128, :], z1b[1],
                                     start=False, stop=True)
                    nc.tensor.matmul(psl1[2][:], w2l1b_t[0:64, :], z1b[2],
                                     start=False, stop=True)
                    for i in range(3):
                        if i == 0:
                            nc.vector.tensor_scalar_add(out_sb[:, 2 + i, :],
                                                        psl1[i][:], 0.0)
                        else:
                            nc.scalar.activation(out_sb[:, 2 + i, :], psl1[i][:],
                                                 AF.Copy)
                    for p in range(2):
                        ps = psum.tile([128, CT], F32, tag="ps_o",
                                       bufs=CFG["ps_o"])
                        nc.tensor.matmul(ps[0:64, :], w2l2_t[:], z2[2 * p][:],
                                         start=True, stop=True,
                                         tile_position=(0, 0))
                        nc.tensor.matmul(ps[64:128, :], w2l2_t[:],
                                         z2[2 * p + 1][:],
                                         start=True, stop=True,
                                         tile_position=(0, 64))
                        nc.scalar.activation(out_sb[:, 5 + p, :], ps[:],
                                             AF.Copy)
                    ps = psum.tile([64, CT], F32, tag="ps_o", bufs=CFG["ps_o"])
                    nc.tensor.matmul(ps[:], w2l2_t[:], z2[4][:],
                                     start=True, stop=True)
                    nc.scalar.activation(out_sb[0:64, 7, :], ps[:], AF.Copy)

                    dst = out[0:896, n0:n0 + CT].rearrange(
                        '(b p) n -> p b n', p=128)
                    nc.gpsimd.dma_start(dst, out_sb[:, 0:7, :])
                    nc.gpsimd.dma_start(out[896:960, n0:n0 + CT],
                                        out_sb[0:64, 7, :])


# ---------------------------------------------------------------------------
# host-side prep + execution
# ---------------------------------------------------------------------------

def _prep_inputs(node_input, node_attr, w1_s, b1_s, w1_l1, w1_l2, w2_s, b2_s,
                 w2_l1, w2_l2):
    a = np.asarray(node_attr, dtype=np.float32)[:, 0]
    attr = None if np.all(a == 1.0) else a
    x = np.asarray(node_input, dtype=np.float32)
    if attr is not None:
        x = x * a[:, None]

    w1s_ = (np.asarray(w1_s, np.float32) / np.sqrt(256.0))
    b1_ = np.asarray(b1_s, dtype=np.float32).reshape(672, 1).copy()
    b1_[384:] *= 0.5  # gate bias halved: gates use tanh(v/2)
    w1sf = w1s_[:, 0:384].astype(np.float16)
    # gate cols packed: 128 g_l1a | 64+64 g_l1b dup | 96 g_l2 | 32 pad
    w1gp = np.zeros((256, 384), np.float32)
    w1gp[:, 0:128] = w1s_[:, 384:512]
    w1gp[:, 128:192] = w1s_[:, 512:576]
    w1gp[:, 192:256] = w1s_[:, 512:576]
    w1gp[:, 256:352] = w1s_[:, 576:672]
    w1sg = np.stack([w1gp[0:128], w1gp[128:256]], axis=1).astype(E4np)
    b1p = np.zeros((768, 1), np.float32)
    b1p[0:384] = b1_[0:384]
    b1p[384:512] = b1_[384:512]
    b1p[512:576] = b1_[512:576]
    b1p[576:640] = b1_[512:576]
    b1p[640:736] = b1_[576:672]

    w1l1 = (np.asarray(w1_l1, np.float32) / np.sqrt(128.0)).astype(np.float16)
    w1l2_ = (np.asarray(w1_l2, np.float32) / np.sqrt(64.0)).astype(np.float16)
    w1l2 = np.concatenate([w1l2_, w1l2_], axis=0)  # rows dup for both PE halves
    w2s = (np.asarray(w2_s, np.float32) / np.sqrt(384.0)).astype(np.float16)
    # l>0 second-layer weights get an extra /2: z_dev = (tanh(v/2)+1)*y = 2*z
    w2l1 = (np.asarray(w2_l1, np.float32) / np.sqrt(192.0) / 2.0).astype(np.float16)
    w2l2 = (np.asarray(w2_l2, np.float32) / np.sqrt(96.0) / 2.0).astype(np.float16)

    in_maps = []
    for c in range(N_CORES):
        xs = x[c * NPC:(c + 1) * NPC, :]  # (NPC, 960)
        xtc = np.empty((960, NPC), dtype=np.float32)
        xtc[0:256] = xs[:, 0:256].T
        for i in range(3):
            xtc[256 + 128 * i:256 + 128 * (i + 1)] = xs[:, 256 + i:640:3].T
        for i in range(5):
            xtc[640 + 64 * i:640 + 64 * (i + 1)] = xs[:, 640 + i:960:5].T
        x0e4 = np.stack([xtc[0:128], xtc[128:256]], axis=1).astype(E4np)
        in_maps.append({
            "xt": xtc.astype(E3np), "x0e4": x0e4,
            "w1sf": w1sf, "w1sg": w1sg, "b1": b1p, "w1l1": w1l1, "w1l2": w1l2,
            "w2s": w2s, "w2l1": w2l1, "w2l2": w2l2,
        })
    return in_maps, attr


def _postprocess(out_full, attr, b2_s):
    b2 = np.asarray(b2_s, dtype=np.float32)
    if attr is not None:
        out_full[:, :256] = out_full[:, :256] * attr[:, None] + b2
        out_full[:, 256:] *= attr[:, None]
    else:
        out_full[:, :256] += b2
    return out_full


_PROGRAM_CACHE = {}


def get_program(npc=NPC, rep=1):
    key = (npc, rep)
    if key not in _PROGRAM_CACHE:
        _PROGRAM_CACHE[key] = build_program(npc=npc, rep=rep)
    return _PROGRAM_CACHE[key]


def kernel(node_input, node_attr, w1_s, b1_s, w1_l1, w1_l2, w2_s, b2_s,
           w2_l1, w2_l2):
    in_maps, attr = _prep_inputs(node_input, node_attr, w1_s, b1_s, w1_l1,
                                 w1_l2, w2_s, b2_s, w2_l1, w2_l2)
    nc = get_program()
    res = run_bass_kernel_spmd(nc, in_maps, list(range(N_CORES)))
    parts = []
    for c in range(N_CORES):
        oc = res.results[c]["out"].astype(np.float32)
        o0 = oc[0:256].T
        o1 = oc[256:640].reshape(3, 128, NPC).transpose(2, 1, 0).reshape(NPC, 384)
        o2 = oc[640:960].reshape(5, 64, NPC).transpose(2, 1, 0).reshape(NPC, 320)
        parts.append(np.concatenate([o0, o1, o2], axis=1))
    out_full = np.concatenate(parts, axis=0)
    return _postprocess(out_full, attr, b2_s)


# revision 4
# speedup vs baseline: 1.1043x; 1.1043x over previous
"""Trainium2 Bass kernel for the gated equivariant MLP (gnn_message_passing).

Baseline structure (single-bank psums, proven scheduler concurrency) with
surgical upgrades:
- gate columns of fctp1's scalar path run as 3 fp8e4 DoubleRow matmuls
  (K=256 contracted at 2 rows/cell/cycle) instead of 6 bf16 matmuls;
  silu columns stay f16 x e3m4 (precision-critical, straight to o0).
- inputs are fp8: x (channel-major, de-interleaved) ships as e3m4 for the
  f16-stationary matmuls, plus an e4m3 copy of the 0e block for DoubleRow.
  Input DMA bytes drop 2x vs bf16.
- all other operands/intermediates/outputs fp16 (2^-11 mantissa) instead of
  bf16, reclaiming error budget for the fp8 stages.
- o0's bias is added on host; the out tensor is fp16 channel-major and the
  host re-transposes/interleaves (host time is off the device critical path).
- gates are computed as t=tanh(v/2) (same ACT table set as silu); z=(t+1)*y
  with host-halved fctp2 l-weights reconstructs sigmoid gating.
"""

import sys

import numpy as np
import ml_dtypes

for _p in ("/root/.axon_site/_ro/trn_rl_repo", "/root/.axon_site/_ro/pypackages",
           "/opt/trn_rl_repo", "/opt/pypackages"):
    if _p not in sys.path:
        sys.path.append(_p)

import concourse.bass as bass
import concourse.bacc as bacc
import concourse.tile as tile
from concourse import mybir
from concourse.bass_utils import run_bass_kernel_spmd

F32 = mybir.dt.float32
F16 = mybir.dt.float16
E4 = mybir.dt.float8e4
E3 = mybir.dt.float8e3
DRM = mybir.MatmulPerfMode.DoubleRow

E4np = ml_dtypes.float8_e4m3
E3np = ml_dtypes.float8_e3m4

N_CORES = 8
N_TOTAL = 65536
NPC = N_TOTAL // N_CORES  # nodes per core

CT = 512   # compute node tile (moving free dim / PSUM bank)
DT = 1024  # input DMA node tile

CFG = {"xin": 3, "mid": 3, "outp": 3, "ps_s": 2, "ps_y": 3, "ps_o": 3}


def build_program(npc=NPC, rep=1, num_devices=N_CORES, sim_safe=False,
                  loop_n=1, variant='full'):
    nc = bacc.Bacc("TRN2", target_bir_lowering=False, debug=False,
                   num_devices=num_devices)

    xt = nc.dram_tensor("xt", [960, npc], E3, kind="ExternalInput").ap()
    x0e4_d = nc.dram_tensor("x0e4", [128, 2, npc], E4, kind="ExternalInput").ap()
    w1sf_d = nc.dram_tensor("w1sf", [256, 384], F16, kind="ExternalInput").ap()
    w1sg_d = nc.dram_tensor("w1sg", [128, 2, 384], E4, kind="ExternalInput").ap()
    b1_d = nc.dram_tensor("b1", [768, 1], F32, kind="ExternalInput").ap()
    w1l1_d = nc.dram_tensor("w1l1", [128, 192], F16, kind="ExternalInput").ap()
    w1l2_d = nc.dram_tensor("w1l2", [128, 96], F16, kind="ExternalInput").ap()
    w2s_d = nc.dram_tensor("w2s", [384, 256], F16, kind="ExternalInput").ap()
    w2l1_d = nc.dram_tensor("w2l1", [192, 128], F16, kind="ExternalInput").ap()
    w2l2_d = nc.dram_tensor("w2l2", [96, 64], F16, kind="ExternalInput").ap()
    out = nc.dram_tensor("out", [960, npc], F16, kind="ExternalOutput").ap()

    with tile.TileContext(nc) as tc:
        if loop_n > 1:
            with tc.For_i(0, loop_n, 1,
                          hint_engines=(mybir.EngineType.PE,
                                        mybir.EngineType.Activation,
                                        mybir.EngineType.DVE,
                                        mybir.EngineType.SP,
                                        mybir.EngineType.Pool)):
                _emit(tc, nc, xt, x0e4_d, w1sf_d, w1sg_d, b1_d, w1l1_d,
                      w1l2_d, w2s_d, w2l1_d, w2l2_d, out, npc, rep)
        else:
            _emit(tc, nc, xt, x0e4_d, w1sf_d, w1sg_d, b1_d, w1l1_d,
                  w1l2_d, w2s_d, w2l1_d, w2l2_d, out, npc, rep)

    nc.compile()
    return nc


def _emit(tc, nc, xt, x0e4_d, w1sf_d, w1sg_d, b1_d, w1l1_d, w1l2_d,
          w2s_d, w2l1_d, w2l2_d, out, npc, rep):
    import contextlib
    ctx = contextlib.ExitStack()
    AF = mybir.ActivationFunctionType
    ADD = mybir.AluOpType.add
    MUL = mybir.AluOpType.mult
    with ctx:
        consts = ctx.enter_context(tc.tile_pool(name="consts", bufs=1))
        xin = ctx.enter_context(tc.tile_pool(name="xin", bufs=CFG["xin"]))
        mid = ctx.enter_context(tc.tile_pool(name="mid", bufs=CFG["mid"]))
        outp = ctx.enter_context(tc.tile_pool(name="outp", bufs=CFG["outp"]))
        psum = ctx.enter_context(tc.tile_pool(name="psum", bufs=2, space="PSUM"))

        # ---- constants into SBUF (once) ----
        w1sf_t = []
        for kb in range(2):
            t = consts.tile([128, 384], F16, tag=f"w1sf{kb}")
            nc.sync.dma_start(t[:], w1sf_d[kb * 128:(kb + 1) * 128, :])
            w1sf_t.append(t)
        w1sg_t = consts.tile([128, 2, 384], E4, tag="w1sg")
        nc.sync.dma_start(w1sg_t[:], w1sg_d[:, :, :])
        b1_t = []
        for mb in range(6):
            t = consts.tile([128, 1], F32, tag=f"b1_{mb}")
            nc.sync.dma_start(t[:], b1_d[mb * 128:(mb + 1) * 128, :])
            b1_t.append(t)
        w1l1_t = consts.tile([128, 192], F16, tag="w1l1")
        nc.sync.dma_start(w1l1_t[:], w1l1_d[:, :])
        w1l2_t = consts.tile([128, 96], F16, tag="w1l2")
        nc.sync.dma_start(w1l2_t[:], w1l2_d[:, :])
        w2s_t = []
        for kb in range(3):
            t = consts.tile([128, 256], F16, tag=f"w2s{kb}")
            nc.sync.dma_start(t[:], w2s_d[kb * 128:(kb + 1) * 128, :])
            w2s_t.append(t)
        w2l1a_t = consts.tile([128, 128], F16, tag="w2l1a")
        nc.sync.dma_start(w2l1a_t[:], w2l1_d[0:128, :])
        w2l1b_t = consts.tile([128, 128], F16, tag="w2l1b")
        nc.sync.dma_start(w2l1b_t[0:64, :], w2l1_d[128:192, :])
        nc.sync.dma_start(w2l1b_t[64:128, :], w2l1_d[128:192, :])
        w2l2_t = consts.tile([96, 64], F16, tag="w2l2")
        nc.sync.dma_start(w2l2_t[:], w2l2_d[:, :])

        n_dt = npc // DT
        n_ct_per_dt = DT // CT

        for _r in range(rep):
            for idt in range(n_dt):
                d0 = idt * DT
                # ---- input DMA (e3m4 main + e4m3 x0 copy) ----
                xa = xin.tile([128, 7, DT], E3, tag="xa")
                nc.sync.dma_start(
                    xa[:], xt[0:896, d0:d0 + DT].rearrange(
                        '(b p) n -> p b n', p=128))
                xbt = xin.tile([64, DT], E3, tag="xb7")
                nc.sync.dma_start(xbt[:], xt[896:960, d0:d0 + DT])
                x0e4_t = xin.tile([128, 2, DT], E4, tag="x0e4")
                nc.sync.dma_start(x0e4_t[:], x0e4_d[:, :, d0:d0 + DT])
                xb = [xa[:, cb, :] for cb in range(7)] + [xbt[:]]
                # x2 component i -> (tile, partition base)
                x2map = [(xb[5], 0), (xb[5], 64), (xb[6], 0), (xb[6], 64), (xb[7], 0)]

                for ict in range(n_ct_per_dt):
                    ns = slice(ict * CT, (ict + 1) * CT)
                    n0 = d0 + ict * CT

                    # ---- fctp1 scalar path + gate nonlinearities ----
                    sc_t = []   # 3x [128, CT] f16 silu outputs
                    g_t = []    # 3x [128, CT] f16 tanh(v/2) gates (g2: rows 0:96)
                    for bi in range(3):   # silu blocks, f16 x e3m4, K=256
                        ps = psum.tile([128, CT], F32, tag="ps_s", bufs=CFG["ps_s"])
                        c0 = bi * 128
                        for kb in range(2):
                            nc.tensor.matmul(
                                ps[:], w1sf_t[kb][:, c0:c0 + 128], xb[kb][:, ns],
                                start=(kb == 0), stop=(kb == 1))
                        dst = mid.tile([128, CT], F16, tag=f"sg{bi}")
                        nc.scalar.activation(dst[:], ps[:], AF.Silu,
                                             bias=b1_t[bi][:])
                        sc_t.append(dst)
                    for gb in range(3):   # gate blocks, fp8e4 DoubleRow, K=256
                        ps = psum.tile([128, CT], F32, tag="ps_s", bufs=CFG["ps_s"])
                        nc.tensor.matmul(
                            ps[:], w1sg_t[:, :, gb * 128:(gb + 1) * 128],
                            x0e4_t[:, :, ns], start=True, stop=True,
                            perf_mode=DRM)
                        dst = mid.tile([128, CT], F16, tag=f"sg{3 + gb}")
                        # t = tanh(v/2); host pre-halved the gate bias rows
                        nc.scalar.activation(dst[:], ps[:], AF.Tanh,
                                             bias=b1_t[3 + gb][:], scale=0.5)
                        g_t.append(dst)

                    # ---- fctp1 l=1, l=2 paths + gating: z = (t+1)*y ----
                    one = 1.0
                    z1a, z1b, z2 = [], [], []
                    for i in range(3):
                        ps = psum.tile([128, CT], F32, tag="ps_y", bufs=CFG["ps_y"])
                        nc.tensor.matmul(ps[:], w1l1_t[:, 0:128], xb[2 + i][:, ns],
                                         start=True, stop=True)
                        z = mid.tile([128, CT], F16, tag=f"z1a{i}")
                        nc.vector.scalar_tensor_tensor(
                            z[:], g_t[0][:], one, ps[:],
                            op0=ADD, op1=MUL)
                        z1a.append(z)
                    psb = psum.tile([128, CT], F32, tag="ps_y", bufs=CFG["ps_y"])
                    nc.tensor.matmul(psb[0:64, :], w1l1_t[:, 128:192],
                                     xb[2][:, ns], start=True, stop=True,
                                     tile_position=(0, 0))
                    nc.tensor.matmul(psb[64:128, :], w1l1_t[:, 128:192],
                                     xb[3][:, ns], start=True, stop=True,
                                     tile_position=(0, 64))
                    ps2b = psum.tile([64, CT], F32, tag="ps_y", bufs=CFG["ps_y"])
                    nc.tensor.matmul(ps2b[:], w1l1_t[:, 128:192], xb[4][:, ns],
                                     start=True, stop=True)
                    z1bp = mid.tile([128, CT], F16, tag="z1bp")
                    nc.vector.scalar_tensor_tensor(
                        z1bp[:], g_t[1][:], one, psb[:],
                        op0=ADD, op1=MUL)
                    z1b2 = mid.tile([64, CT], F16, tag="z1b2")
                    nc.vector.scalar_tensor_tensor(
                        z1b2[:], g_t[1][0:64, :], one, ps2b[:],
                        op0=ADD, op1=MUL)
                    z1b = [z1bp[0:64, :], z1bp[64:128, :], z1b2[:]]
                    for i in range(5):
                        xt2, p0 = x2map[i]
                        ps = psum.tile([96, CT], F32, tag="ps_y", bufs=CFG["ps_y"])
                        nc.tensor.matmul(ps[:], w1l2_t[p0:p0 + 64, :],
                                         xt2[p0:p0 + 64, ns], start=True, stop=True)
                        z = mid.tile([96, CT], F16, tag=f"z2{i}")
                        nc.vector.scalar_tensor_tensor(
                            z[:], g_t[2][0:96, :], one, ps[:],
                            op0=ADD, op1=MUL)
                        z2.append(z)

                    # ---- fctp2 (weight-stationary -> channel-major out) ----
                    out_sb = outp.tile([128, 8, CT], F16, tag="out_sb")
                    for ob in range(2):
                        ps = psum.tile([128, CT], F32, tag="ps_o", bufs=CFG["ps_o"])
                        obs = slice(ob * 128, (ob + 1) * 128)
                        for kb in range(3):
                            nc.tensor.matmul(ps[:], w2s_t[kb][:, obs], sc_t[kb][:],
                                             start=(kb == 0), stop=(kb == 2))
                        nc.scalar.activation(out_sb[:, ob, :], ps[:], AF.Copy)
                    psl1 = []
                    for i in range(3):
                        ps = psum.tile([128, CT], F32, tag="ps_o", bufs=CFG["ps_o"])
                        nc.tensor.matmul(ps[:], w2l1a_t[:], z1a[i][:],
                                         start=True, stop=False)
                        psl1.append(ps)
                    nc.tensor.matmul(psl1[0][:], w2l1b_t[0:64, :], z1b[0],
                                     start=False, stop=True)
                    nc.tensor.matmul(psl1[1][:], w2l1b_t[64:128, :], z1b[1],
                                     start=False, stop=True)
                    nc.tensor.matmul(psl1[2][:], w2l1b_t[0:64, :], z1b[2],
                                     start=False, stop=True)
                    for i in range(3):
                        if i == 0:
                            nc.vector.tensor_scalar_add(out_sb[:, 2 + i, :],
                                                        psl1[i][:], 0.0)
                        else:
                            nc.scalar.activation(out_sb[:, 2 + i, :], psl1[i][:],
                                                 AF.Copy)
                    for p in range(2):
                        ps = psum.tile([128, CT], F32, tag="ps_o",
                                       bufs=CFG["ps_o"])
                        nc.tensor.matmul(ps[0:64, :], w2l2_t[:], z2[2 * p][:],
                                         start=True, stop=True,
                                         tile_position=(0, 0))
                        nc.tensor.matmul(ps[64:128, :], w2l2_t[:],
                                         z2[2 * p + 1][:],
                                         start=True, stop=True,
                                         tile_position=(0, 64))
                        nc.scalar.activation(out_sb[:, 5 + p, :], ps[:],
                                             AF.Copy)
                    ps = psum.tile([64, CT], F32, tag="ps_o", bufs=CFG["ps_o"])
                    nc.tensor.matmul(ps[:], w2l2_t[:], z2[4][:],
                                     start=True, stop=True)
                    nc.scalar.activation(out_sb[0:64, 7, :], ps[:], AF.Copy)

                    dst = out[0:896, n0:n0 + CT].rearrange(
                        '(b p) n -> p b n', p=128)
                    nc.gpsimd.dma_start(dst, out_sb[:, 0:7, :])
                    nc.gpsimd.dma_start(out[896:960, n0:n0 + CT],
                                        out_sb[0:64, 7, :])


# ---------------------------------------------------------------------------
# host-side prep + execution
# ---------------------------------------------------------------------------

def _prep_inputs(node_input, node_attr, w1_s, b1_s, w1_l1, w1_l2, w2_s, b2_s,
                 w2_l1, w2_l2):
    a = np.asarray(node_attr, dtype=np.float32)[:, 0]
    attr = None if np.all(a == 1.0) else a
    x = np.asarray(node_input, dtype=np.float32)
    if attr is not None:
        x = x * a[:, None]

    w1s_ = (np.asarray(w1_s, np.float32) / np.sqrt(256.0))
    b1_ = np.asarray(b1_s, dtype=np.float32).reshape(672, 1).copy()
    b1_[384:] *= 0.5  # gate bias halved: gates use tanh(v/2)
    w1sf = w1s_[:, 0:384].astype(np.float16)
    # gate cols packed: 128 g_l1a | 64+64 g_l1b dup | 96 g_l2 | 32 pad
    w1gp = np.zeros((256, 384), np.float32)
    w1gp[:, 0:128] = w1s_[:, 384:512]
    w1gp[:, 128:192] = w1s_[:, 512:576]
    w1gp[:, 192:256] = w1s_[:, 512:576]
    w1gp[:, 256:352] = w1s_[:, 576:672]
    w1sg = np.stack([w1gp[0:128], w1gp[128:256]], axis=1).astype(E4np)
    b1p = np.zeros((768, 1), np.float32)
    b1p[0:384] = b1_[0:384]
    b1p[384:512] = b1_[384:512]
    b1p[512:576] = b1_[512:576]
    b1p[576:640] = b1_[512:576]
    b1p[640:736] = b1_[576:672]

    w1l1 = (np.asarray(w1_l1, np.float32) / np.sqrt(128.0)).astype(np.float16)
    w1l2_ = (np.asarray(w1_l2, np.float32) / np.sqrt(64.0)).astype(np.float16)
    w1l2 = np.concatenate([w1l2_, w1l2_], axis=0)  # rows dup for both PE halves
    w2s = (np.asarray(w2_s, np.float32) / np.sqrt(384.0)).astype(np.float16)
    # l>0 second-layer weights get an extra /2: z_dev = (tanh(v/2)+1)*y = 2*z
    w2l1 = (np.asarray(w2_l1, np.float32) / np.sqrt(192.0) / 2.0).astype(np.float16)
    w2l2 = (np.asarray(w2_l2, np.float32) / np.sqrt(96.0) / 2.0).astype(np.float16)

    in_maps = []
    for c in range(N_CORES):
        xs = x[c * NPC:(c + 1) * NPC, :]  # (NPC, 960)
        xtc = np.empty((960, NPC), dtype=np.float32)
        xtc[0:256] = xs[:, 0:256].T
        for i in range(3):
            xtc[256 + 128 * i:256 + 128 * (i + 1)] = xs[:, 256 + i:640:3].T
        for i in range(5):
            xtc[640 + 64 * i:640 + 64 * (i + 1)] = xs[:, 640 + i:960:5].T
        x0e4 = np.stack([xtc[0:128], xtc[128:256]], axis=1).astype(E4np)
        in_maps.append({
            "xt": xtc.astype(E3np), "x0e4": x0e4,
            "w1sf": w1sf, "w1sg": w1sg, "b1": b1p, "w1l1": w1l1, "w1l2": w1l2,
            "w2s": w2s, "w2l1": w2l1, "w2l2": w2l2,
        })
    return in_maps, attr


def _postprocess(out_full, attr, b2_s):
    b2 = np.asarray(b2_s, dtype=np.float32)
    if attr is not None:
        out_full[:, :256] = out_full[:, :256] * attr[:, None] + b2
        out_full[:, 256:] *= attr[:, None]
    else:
        out_full[:, :256] += b2
    return out_full


_PROGRAM_CACHE = {}


def get_program(npc=NPC, rep=1):
    key = (npc, rep)
    if key not in _PROGRAM_CACHE:
        _PROGRAM_CACHE[key] = build_program(npc=npc, rep=rep)
    return _PROGRAM_CACHE[key]


def kernel(node_input, node_attr, w1_s, b1_s, w1_l1, w1_l2, w2_s, b2_s,
           w2_l1, w2_l2):
    in_maps, attr = _prep_inputs(node_input, node_attr, w1_s, b1_s, w1_l1,
                                 w1_l2, w2_s, b2_s, w2_l1, w2_l2)
    nc = get_program()
    res = run_bass_kernel_spmd(nc, in_maps, list(range(N_CORES)))
    parts = []
    for c in range(N_CORES):
        oc = res.results[c]["out"].astype(np.float32)
        o0 = oc[0:256].T
        o1 = oc[256:640].reshape(3, 128, NPC).transpose(2, 1, 0).reshape(NPC, 384)
        o2 = oc[640:960].reshape(5, 64, NPC).transpose(2, 1, 0).reshape(NPC, 320)
        parts.append(np.concatenate([o0, o1, o2], axis=1))
    out_full = np.concatenate(parts, axis=0)
    return _postprocess(out_full, attr, b2_s)
